# revision 41
# baseline (speedup 1.0000x reference)
"""Trainium2 Bass kernel for nn_Attention_83141976916236.

Reference computation (B=2, N=2048, C=512, H=8, D=64):
    qkv = x @ qkv_w                       -> split to q, k, v per head
    att_h = softmax(q_h k_h^T / sqrt(D)) v_h        (per batch b, head h)
    out  = reshape_no_transpose(att) @ proj_w + proj_b

Key structural fact: the reference reshapes (B,H,N,D) -> (B,N,C) WITHOUT
transposing, so output row n' = h*256 + n//8 with channel c' = (n%8)*64 + d.
Every output row therefore depends on exactly ONE head: with heads sharded
across cores, each core produces a disjoint slice of output rows and the
host-side unshard is a pure concatenation (no cross-core reduction).

Sharding (8 cores): core c handles batch b = c//4 and heads (2p, 2p+1) where
p = c%4. Per core, everything runs in fp16 on the PE (fp32 PSUM accumulate):

- QKV: explicit 128-col ldweights shared by two 512-wide moving chunks.
- Scores: ONE full 128x128 ldweights loads BOTH heads' K^T block (head h
  occupies array rows 64h..64h+63); the two per-head matmuls then address
  disjoint 64-row PE tiles (tile_position (0,0)/(64,0)) and execute
  CONCURRENTLY on the systolic array, halving score time vs per-head loads.
- Softmax: scoresT kept [j, i]; exp on ACT (table set natural_log_exp: Exp,
  Ln and Copy live in one set so no ACT table reloads). An appended ones
  column in the AV weights yields the denominator row for free. Denominators
  are evacuated PSUM->SBUF immediately so the next i-chunk's AV accumulation
  can reuse the PSUM bank without waiting for the (slow, off-critical-path)
  reciprocal: iq 0-2 reciprocals on DVE, the final iq's on ACT via
  exp(-ln d) right after the last exp call. The reciprocal row is broadcast
  across 64 partitions by a tiny fp16 ones-vector matmul.
- Projection: attn (fp16) slices are the stationary operand with explicit
  ldweights; heads 0/1 interleave on disjoint 64-row PE tiles so their
  matmul streams overlap. (The old fp32r self-loading path cost ~8x more.)
- Optional knob exp_dve_jbs routes the exp of selected j-blocks to the DVE
  as a Schraudolph fp16 bit-trick (tensor_scalar mult+add -> int16 view);
  ~3.6% elementwise sawtooth error that largely averages out in AV.

Emission is software-pipelined one group (iq i-chunk, jb j-block) ahead:
scores(g+1) are issued before AV(g) so the PE computes scores while ACT
evaluates exp(g). The back half of QKV is spread into the early attention
stream; projections fire mid-stream (mb=0) and at the tail (mb=1).

Host-side prep per core: x[b] transposed to channel-major (the PE contracts
over the partition axis), qkv_w column slice for its heads, proj_w
rearranged fp16 for the scrambled-row projection. Host-side unshard:
row-slice concatenation + bias add.
"""

import numpy as np
import ml_dtypes
from contextlib import ExitStack

import concourse.tile as tile
from concourse import bacc, mybir
from concourse.bass_utils import run_bass_kernel_spmd
from concourse.masks import make_identity

B, N, C, H = 2, 2048, 512, 8
D = C // H            # 64
SCALE = D ** -0.5
N_CORES = 8
F32 = mybir.dt.float32
FP16 = mybir.dt.float16
I16 = mybir.dt.int16
EXP = mybir.ActivationFunctionType.Exp
LN = mybir.ActivationFunctionType.Ln
COPY = mybir.ActivationFunctionType.Copy

# Schraudolph fp16 exp: bits = round(s*SCALE * 1024/ln2 + (15360 + C_ADJ))
EXPA = SCALE * 1024.0 / float(np.log(2.0))
EXPB = 15360.0 - 58.0

_programs = {}


def build_program(reps: int = 1, debug: bool = False,
                  do_attn: bool = True, do_proj: bool = True,
                  do_qkv: bool = True, exp_dve_jbs: tuple = (),
                  expb: float = EXPB, recip_act_all: bool = False,
                  exp_half: bool = False, scores_single: bool = False,
                  unroll: int = 1, loop_kw: dict | None = None):
    """Build + compile the SPMD single-core program.

    reps > 1 wraps the whole body in a hardware loop (used only for timing
    calibration). debug=True adds DRAM dumps of intermediates. The do_* /
    exp_dve_jbs knobs build timing-experiment variants.
    """
    nc = bacc.Bacc("TRN2", target_bir_lowering=False, debug=False,
                   num_devices=N_CORES)
    xt = nc.dram_tensor("xt", [C, N], FP16, kind="ExternalInput").ap()
    wqkv = nc.dram_tensor("wqkv", [C, 384], FP16, kind="ExternalInput").ap()
    wp = nc.dram_tensor("wp", [128, 8, C], FP16, kind="ExternalInput").ap()
    part = nc.dram_tensor("part", [512, C], F32, kind="ExternalOutput").ap()
    dbg = {}
    if debug:
        for name, shape in (("d_qT", [128, N]), ("d_kT", [128, N]),
                            ("d_vext", [128, 16 * 130]), ("d_attn", [128, N])):
            dbg[name] = nc.dram_tensor(name, shape, F32, kind="ExternalOutput").ap()

    with tile.TileContext(nc) as tc, ExitStack() as ctx:
        ctx.enter_context(nc.allow_low_precision(reason="fp16 attention kernel"))
        consts = ctx.enter_context(tc.tile_pool(name="consts", bufs=1))
        bigs = ctx.enter_context(tc.tile_pool(name="bigs", bufs=1))
        probs_pool = ctx.enter_context(tc.tile_pool(name="probs", bufs=4))
        small = ctx.enter_context(tc.tile_pool(name="small", bufs=2))
        outp = ctx.enter_context(tc.tile_pool(name="outp", bufs=2))

        ident_f = consts.tile([128, 128], F32)
        make_identity(nc, ident_f[:])
        ident16 = consts.tile([128, 128], FP16)
        nc.vector.tensor_copy(out=ident16[:], in_=ident_f[:])
        ones_f = consts.tile([128, 128], F32)
        nc.vector.memset(ones_f[:], 1.0)
        ones16 = consts.tile([1, 128], FP16)
        nc.vector.tensor_copy(out=ones16[:], in_=ones_f[0:1, :])
        ones_wide = consts.tile([128, 32], FP16)
        nc.vector.tensor_copy(out=ones_wide[:], in_=ones_f[:, 0:32])

        def body():
            # ---- loads -------------------------------------------------
            # weights first (small), then x in 4 n-chunks so the first QKV
            # matmuls start early instead of waiting for the full 2MB.
            wqkv_sb = bigs.tile([128, 4, 384], FP16, tag="wqkv")
            wqkv_v = wqkv.rearrange("(k p) f -> p k f", p=128)
            for k in range(4):
                nc.sync.dma_start(out=wqkv_sb[:, k, :], in_=wqkv_v[:, k, :])
            xt_sb = bigs.tile([128, 4, 4, 512], FP16, tag="xt")
            xt_v = xt.rearrange("(k p) (nb n) -> p k nb n", p=128, nb=4)
            for k in range(4):
                nc.sync.dma_start(out=xt_sb[:, k, 0:2, :], in_=xt_v[:, k, 0:2, :])
            for nb in range(2, 4):
                nc.sync.dma_start(out=xt_sb[:, :, nb, :], in_=xt_v[:, :, nb, :])
            wp_sb = bigs.tile([128, 8, C], FP16, tag="wp")
            nc.sync.dma_start(out=wp_sb[:], in_=wp)

            qT = bigs.tile([128, N], FP16, tag="qT")
            kT = bigs.tile([128, N], FP16, tag="kT")
            vT = bigs.tile([128, N], FP16, tag="vT")
            attn = bigs.tile([128, N], FP16, tag="attn")
            # v in row-major [j, 64+ones | 64+ones] blocks; ones col feeds the
            # softmax-denominator row of the AV matmul.
            vext = bigs.tile([128, 16, 130], FP16, tag="vext")
            vext_cols = vext[:].rearrange("p a (b c) -> p a b c", b=2)
            nc.vector.tensor_copy(
                out=vext_cols[:, :, :, 64],
                in_=ones_wide[:].rearrange("p (a b) -> p a b", a=16))

            # PSUM: scr 2 banks x2 bufs + av0/av1 1 bank each + qkv 2 = 8.
            with tc.tile_pool(name="ps_qkv", bufs=2, space="PSUM") as ps_qkv, \
                 tc.tile_pool(name="ps_scr", bufs=2, space="PSUM") as ps_scr, \
                 tc.tile_pool(name="ps_av", bufs=1, space="PSUM") as ps_av:
                dests = (qT, kT, vT)

                def qkv_f_mm(nb, f, pa, pb, k):
                    w = wqkv_sb[:, k, f * 128:(f + 1) * 128]
                    nc.tensor.ldweights(weights=w)
                    for ps, nbx in ((pa, nb), (pb, nb + 1)):
                        mm = nc.tensor.matmul(
                            ps[:], w, xt_sb[:, k, nbx, :],
                            start=(k == 0), stop=(k == 3))
                        mm.ins.ldweights = False

                def qkv_f_copy(nb, f, pa, pb, copy_eng):
                    for ps, nbx in ((pa, nb), (pb, nb + 1)):
                        dst = dests[f][:, nbx * 512:(nbx + 1) * 512]
                        if copy_eng == "act":
                            nc.scalar.activation(out=dst, in_=ps[:], func=COPY)
                        else:
                            nc.vector.tensor_copy(out=dst, in_=ps[:])

                def qkv_fs(nb, fs, copy_eng):
                    if not do_qkv and nb == 0 and 0 in fs:
                        for t in dests:
                            nc.vector.memset(t[:], 0.00390625)
                    # two n-chunks share each fp16 weight load (LDW reuse)
                    for f in (fs if do_qkv else ()):
                        pa = ps_qkv.tile([128, 512], F32, tag="qkv", name=f"qa{nb}{f}")
                        pb = ps_qkv.tile([128, 512], F32, tag="qkv", name=f"qb{nb}{f}")
                        for k in range(4):
                            qkv_f_mm(nb, f, pa, pb, k)
                        qkv_f_copy(nb, f, pa, pb, copy_eng)

                def v_transpose(jb):
                    pst = ps_qkv.tile([128, 128], FP16, tag="qkv")
                    nc.tensor.transpose(pst[:], vT[:, jb * 128:(jb + 1) * 128], ident16[:])
                    nc.vector.tensor_copy(out=vext[:, jb, 0:64], in_=pst[:, 0:64])
                    nc.vector.tensor_copy(out=vext[:, jb, 65:129], in_=pst[:, 64:128])

                def v_transposes(nb):
                    # transpose these n-chunks of v to row-major via PE
                    for jb in range(4 * nb, 4 * nb + 8):
                        v_transpose(jb)

                def qkv_pair(nb, copy_eng):
                    qkv_fs(nb, (0, 1, 2), copy_eng)
                    v_transposes(nb)

                def scores_g(iq, jb):
                    # scoresT[j, i] for 128 j's x (2 heads x 512 i's); ONE
                    # full-width ldweights holds both heads' K^T block and
                    # the two matmuls run concurrently on disjoint 64-row
                    # PE tiles.
                    scr = ps_scr.tile([128, 1024], F32, tag="scr")
                    nc.tensor.ldweights(weights=kT[:, jb * 128:(jb + 1) * 128])
                    for h in range(1 if scores_single else 2):
                        hp = slice(64 * h, 64 * h + 64)
                        mm = nc.tensor.matmul(
                            scr[:, h * 512:(h + 1) * 512],
                            kT[hp, jb * 128:(jb + 1) * 128],
                            qT[hp, iq * 512:(iq + 1) * 512],
                            start=True, stop=True)
                        mm.ins.ldweights = False
                    # scores_single is only valid with exp_half (which never
                    # reads scr[:, 512:]): isolates MM_B for concurrency
                    # timing.
                    return scr

                def exp_g(scr, jb):
                    pr = probs_pool.tile([128, 1024], FP16, tag="pr")
                    if jb in exp_dve_jbs:
                        nc.vector.tensor_scalar(
                            out=pr[:].bitcast(I16), in0=scr[:],
                            scalar1=float(EXPA), scalar2=float(expb),
                            op0=mybir.AluOpType.mult, op1=mybir.AluOpType.add)
                    elif exp_half:
                        # timing experiment: half the ACT columns (wrong)
                        nc.scalar.activation(out=pr[:, 0:512], in_=scr[:, 0:512],
                                             func=EXP, scale=SCALE)
                    else:
                        nc.scalar.activation(out=pr[:], in_=scr[:], func=EXP,
                                             scale=SCALE)
                    return pr

                def av_g(h, av, pr, jb):
                    vblk = vext[:, jb, 65 * h:65 * h + 65]
                    nc.tensor.ldweights(weights=vblk)
                    prh = pr[:, 0:512] if exp_half else pr[:, h * 512:(h + 1) * 512]
                    mm = nc.tensor.matmul(
                        av[0:65, :], vblk, prh,
                        start=(jb == 0), stop=(jb == 15))
                    mm.ins.ldweights = False

                def evac(h, iq, av):
                    # evacuate av PSUM->SBUF so the bank frees for the next
                    # iq without waiting for the (slow) reciprocal. Both
                    # heads evacuate BEFORE either reciprocal is emitted so
                    # the in-order DVE never holds av1's bank hostage.
                    avs = small.tile([128, 512], F32, tag=f"avs{h}", bufs=2,
                                     name=f"avs{h}_{iq}")
                    nc.vector.tensor_copy(out=avs[0:65, :], in_=av[0:65, :])
                    return avs

                def recip(h, iq, avs):
                    rc = small.tile([1, 512], FP16, tag=f"rc{h}", bufs=2,
                                    name=f"rc{h}_{iq}")
                    if recip_act_all or iq == 3:
                        ld = small.tile([1, 512], F32, tag=f"ld{h}", bufs=2,
                                        name=f"ld{h}_{iq}")
                        nc.scalar.activation(out=ld[:], in_=avs[64:65, :], func=LN)
                        nc.scalar.activation(out=rc[:], in_=ld[:], func=EXP,
                                             scale=-1.0)
                    else:
                        nc.vector.reciprocal(rc[:], avs[64:65, :])
                    return rc

                def finish_norm(h, iq, avs, rc):
                    if iq == 3:
                        # reuse the (evacuated) av bank: keeps the qkv-tag
                        # rotation free at the tail so the next rep's first
                        # QKV PSUM alloc isn't gated on this rep's norm.
                        bc = ps_av.tile([128, 512], F32, tag=f"av{h}",
                                        name=f"bc{h}{iq}")
                    else:
                        bc = ps_qkv.tile([128, 512], F32, tag="qkv",
                                         name=f"bc{h}{iq}")
                    nc.tensor.matmul(bc[0:64, :], ones16[0:1, 0:64], rc[0:1, :],
                                     start=True, stop=True)
                    # The host permutes x's token axis g-major (position
                    # p = mb*1024 + g*128 + m for token i = mb*1024+8m+g),
                    # so this contiguous write leaves attn exactly in the
                    # layout whose per-(mb,g) projection weight slices are
                    # contiguous 128-col ldweights (fast-weight-load path).
                    nc.vector.tensor_mul(
                        attn[64 * h:64 * h + 64, iq * 512:(iq + 1) * 512],
                        avs[0:64, :], bc[0:64, :])

                def proj_mms(mb, pps, gs):
                    # projection for both heads, interleaved so the per-head
                    # matmuls run concurrently on disjoint 64-row PE tiles.
                    # out rows n'=h*256+mb*128+m, contraction c'=(g,d); attn
                    # is g-major so each weight slice is contiguous.
                    glast = 7 if do_proj else 0
                    for g in gs:
                        for h in range(2):
                            hp = slice(64 * h, 64 * h + 64)
                            w = attn[hp, :].rearrange(
                                "p (mb g m) -> p mb g m", mb=2, g=8)[:, mb, g, :]
                            nc.tensor.ldweights(weights=w, tile_position=(64 * h, 0))
                            mm = nc.tensor.matmul(
                                pps[h][:], w, wp_sb[hp, g, :],
                                start=(g == 0), stop=(g == glast),
                                tile_position=(64 * h, 0))
                            mm.ins.ldweights = False

                def proj_out(mb, pps):
                    for h in range(2):
                        ob = outp.tile([128, 512], F32, tag="ob")
                        nc.vector.tensor_copy(out=ob[:], in_=pps[h][:])
                        # issue the output DMA from the (otherwise idle)
                        # Pool engine: keeping SP's queue load-only lets the
                        # next rep's input DMAs issue during this rep's tail
                        nc.gpsimd.dma_start(
                            out=part.rearrange("(r p) c -> r p c", p=128)[2 * h + mb],
                            in_=ob[:])

                def proj_pair(mb):
                    pps = [ps_qkv.tile([128, 512], F32, tag="qkv",
                                       name=f"pp{h}{mb}") for h in range(2)]
                    proj_mms(mb, pps, range(8 if do_proj else 1))
                    proj_out(mb, pps)

                # Software-pipelined emission: scores of group g+1 are
                # emitted BEFORE av of group g so the static schedule lets
                # the PE run ahead while ACT evaluates exp(g); the back half
                # of QKV is spread into the early attention stream. Norm
                # completion (bc broadcast + divide-multiply) for iq is
                # deferred to mid-(iq+1) so slow reciprocals never stall
                # the in-order PE stream.
                # Dribble plan for the second QKV half: tiny self-contained
                # PE pieces (one ldw+mm, or a transpose, or a PSUM-drain
                # copy) spread through the ACT-bound stream's PE slack.
                # Ordering constraints: v chunks feed transposes; kT chunk
                # nb must be EMITTED before the scores prefetch that reads
                # it (jb8 at idx6, jb12 at idx10); vext jb must be emitted
                # before av reads it (idx jb); q needed by iq2 (idx30).
                drib_units = {}

                def drib_mm(nb, f, k):
                    if (nb, f) not in drib_units:
                        drib_units[(nb, f)] = ps_qkv.tile(
                            [128, 512], F32, tag="qkv", name=f"qu{nb}{f}")
                    pu = drib_units[(nb, f)]
                    w = wqkv_sb[:, k, f * 128:(f + 1) * 128]
                    nc.tensor.ldweights(weights=w)
                    mm = nc.tensor.matmul(pu[:], w, xt_sb[:, k, nb, :],
                                          start=(k == 0), stop=(k == 3))
                    mm.ins.ldweights = False

                def drib_copy(nb, f):
                    pu = drib_units.pop((nb, f))
                    nc.vector.tensor_copy(
                        out=dests[f][:, nb * 512:(nb + 1) * 512], in_=pu[:])

                DRIB = {}
                if do_qkv:
                    def M(nb, f, k):
                        return ("mm", nb, f, k)
                    # Hand-scheduled: dribble(idx) runs at the END of group
                    # idx's emission, so a piece needed by group i's own
                    # reads must sit at idx <= i-1: vext t(jb) before av at
                    # idx jb; kT chunk nb2/nb3 before the scores PREFETCH
                    # (emitted at idx jb-2) that reads it; vT chunks before
                    # their transposes; q chunks before iq2 (idx 30).
                    DRIB = {
                        0: [M(2, 2, 0), M(2, 2, 1)],
                        1: [M(2, 2, 2), M(2, 2, 3)],
                        2: [("cp", 2, 2), ("t", 8)],
                        3: [M(2, 1, 0), M(2, 1, 1)],
                        4: [M(2, 1, 2), M(2, 1, 3)],
                        5: [("cp", 2, 1), ("t", 9)],
                        6: [M(3, 1, 0), M(3, 1, 1)],
                        7: [M(3, 1, 2), M(3, 1, 3)],
                        8: [("cp", 3, 1), ("t", 10)],
                        9: [M(3, 2, 0), M(3, 2, 1), ("t", 11)],
                        10: [M(3, 2, 2), M(3, 2, 3), ("cp", 3, 2)],
                        11: [("t", 12), ("t", 13)],
                        12: [("t", 14), ("t", 15)],
                        13: [M(2, 0, 0), M(2, 0, 1)],
                        14: [M(2, 0, 2), M(2, 0, 3)],
                        15: [("cp", 2, 0)],
                        16: [M(3, 0, 0), M(3, 0, 1)],
                        17: [M(3, 0, 2), M(3, 0, 3)],
                        18: [("cp", 3, 0)],
                    }

                def dribble(idx):
                    for piece in DRIB.get(idx, ()):
                        if piece[0] == "mm":
                            drib_mm(piece[1], piece[2], piece[3])
                        elif piece[0] == "cp":
                            drib_copy(piece[1], piece[2])
                        else:
                            v_transpose(piece[1])

                if do_attn:
                    groups = [(iq, jb) for iq in range(4) for jb in range(16)]
                    navs = {}
                    # prefix: q,k then the first two score groups, then v.
                    # Prefix copies ride ACT: the next rep's stream start
                    # depends on them, and ACT's tail backlog at the rep
                    # boundary (~2.5us) is far shorter than DVE's (~12us).
                    qkv_fs(0, (0, 1), copy_eng="act")
                    scrs = [scores_g(*groups[0]), scores_g(*groups[1])]
                    qkv_fs(0, (2,), copy_eng="act")
                    v_transposes(0)
                    for idx, (iq, jb) in enumerate(groups):
                        if jb == 0:
                            av0 = ps_av.tile([128, 512], F32, tag="av0",
                                             name=f"av0_{iq}")
                            av1 = ps_av.tile([128, 512], F32, tag="av1",
                                             name=f"av1_{iq}")
                        pr = exp_g(scrs[idx], jb)
                        if idx + 2 < len(groups):
                            scrs.append(scores_g(*groups[idx + 2]))
                        av_g(0, av0, pr, jb)
                        av_g(1, av1, pr, jb)
                        dribble(idx)
                        if jb == 15:
                            avs0 = evac(0, iq, av0)
                            avs1 = evac(1, iq, av1)
                            if iq < 3:
                                navs[(0, iq)] = (avs0, recip(0, iq, avs0))
                                navs[(1, iq)] = (avs1, recip(1, iq, avs1))
                        if jb == 11 and iq >= 1:
                            for h in range(2):
                                finish_norm(h, iq - 1, *navs.pop((h, iq - 1)))
                        if (iq, jb) == (2, 13):
                            proj_pair(0)
                        if (iq, jb) == (3, 13) and do_proj:
                            # attn is g-major: proj mb=1's g 0-3 touch only
                            # iq2's rows (normalized at (3,11)) - start the
                            # second projection before the last iq finishes.
                            pps1 = [ps_qkv.tile([128, 512], F32, tag="qkv",
                                                name=f"pp{h}1") for h in range(2)]
                            proj_mms(1, pps1, range(0, 4))
                    # tail: overlap head-0 norm with head-1's reciprocal.
                    rc0 = recip(0, 3, avs0)
                    finish_norm(0, 3, avs0, rc0)
                    rc1 = recip(1, 3, avs1)
                    finish_norm(1, 3, avs1, rc1)
                    if do_proj:
                        proj_mms(1, pps1, range(4, 8))
                        proj_out(1, pps1)
                    else:
                        proj_pair(1)
                else:
                    qkv_pair(0, copy_eng="act")
                    qkv_pair(2, copy_eng="dve")
                    nc.vector.memset(attn[:], 0.00390625)
                    proj_pair(0)
                    proj_pair(1)
            if debug:
                for name, t in (("d_qT", qT), ("d_kT", kT), ("d_attn", attn)):
                    sb = outp.tile([128, N], F32, tag="dbg")
                    nc.vector.tensor_copy(out=sb[:], in_=t[:])
                    nc.sync.dma_start(out=dbg[name], in_=sb[:])
                sb = outp.tile([128, 16 * 130], F32, tag="dbg")
                nc.vector.tensor_copy(out=sb[:], in_=vext[:].rearrange("p a b -> p (a b)"))
                nc.sync.dma_start(out=dbg["d_vext"], in_=sb[:])

        if reps == 1:
            for _ in range(unroll):
                body()
        else:
            assert reps % unroll == 0
            with tc.For_i(0, reps // unroll, 1, **(loop_kw or {})):
                for _ in range(unroll):
                    body()

    nc.compile()
    return nc


def _get_program(reps: int = 1, debug: bool = False, **kw):
    key = (reps, debug, repr(sorted(kw.items())))
    if key not in _programs:
        _programs[key] = build_program(reps, debug, **kw)
    return _programs[key]


def _token_perm():
    """Device token order: position mb*1024 + g*128 + m holds token
    i = mb*1024 + 8m + g. Softmax is order-invariant over j and each
    column's softmax is complete, so permuting the token axis on the host
    makes every device-side access contiguous AND leaves attn g-major so
    projection weight loads hit the fast contiguous ldweights path."""
    i = np.arange(N).reshape(2, 128, 8)            # [mb, m, g]
    return i.transpose(0, 2, 1).reshape(N)         # [mb, g, m] -> flat


def _in_maps(x, qkv_w, proj_w):
    perm = _token_perm()
    wp_arr = np.ascontiguousarray(
        np.tile(proj_w.reshape(8, 64, C).transpose(1, 0, 2),
                (2, 1, 1))).astype(np.float16)
    maps = []
    for c in range(N_CORES):
        b, p = divmod(c, 4)
        xt = np.ascontiguousarray(x[b][perm].T.astype(np.float16))
        wqkv = np.ascontiguousarray(np.concatenate(
            [qkv_w[:, t * C + p * 128: t * C + p * 128 + 128] for t in range(3)],
            axis=1).astype(np.float16))
        maps.append({"xt": xt, "wqkv": wqkv, "wp": wp_arr})
    return maps


def kernel(**inputs) -> np.ndarray:
    x = np.asarray(inputs["x"], np.float32)
    qkv_w = np.asarray(inputs["qkv_w"], np.float32)
    proj_w = np.asarray(inputs["proj_w"], np.float32)
    proj_b = np.asarray(inputs["proj_b"], np.float32)

    nc = _get_program()
    res = run_bass_kernel_spmd(nc, _in_maps(x, qkv_w, proj_w),
                               core_ids=list(range(N_CORES)))
    out = np.empty((B, N, C), np.float32)
    for c in range(N_CORES):
        b, p = divmod(c, 4)
        out[b, p * 512:(p + 1) * 512, :] = res.results[c]["part"]
    out += proj_b
    return out


# revision 42
# speedup vs baseline: 1.0292x; 1.0292x over previous
"""Trainium2 Bass kernel for nn_Attention_83141976916236.

Reference computation (B=2, N=2048, C=512, H=8, D=64):
    qkv = x @ qkv_w                       -> split to q, k, v per head
    att_h = softmax(q_h k_h^T / sqrt(D)) v_h        (per batch b, head h)
    out  = reshape_no_transpose(att) @ proj_w + proj_b

Key structural fact: the reference reshapes (B,H,N,D) -> (B,N,C) WITHOUT
transposing, so output row n' = h*256 + n//8 with channel c' = (n%8)*64 + d.
Every output row therefore depends on exactly ONE head: with heads sharded
across cores, each core produces a disjoint slice of output rows and the
host-side unshard is a pure concatenation (no cross-core reduction).

Sharding (8 cores): core c handles batch b = c//4 and heads (2p, 2p+1) where
p = c%4. Per core, everything runs in fp16 on the PE (fp32 PSUM accumulate):

- QKV: explicit 128-col ldweights shared by two 512-wide moving chunks.
- Scores: ONE full 128x128 ldweights loads BOTH heads' K^T block (head h
  occupies array rows 64h..64h+63); the two per-head matmuls then address
  disjoint 64-row PE tiles (tile_position (0,0)/(64,0)) and execute
  CONCURRENTLY on the systolic array, halving score time vs per-head loads.
- Softmax: scoresT kept [j, i]; exp on ACT (table set natural_log_exp: Exp,
  Ln and Copy live in one set so no ACT table reloads). An appended ones
  column in the AV weights yields the denominator row for free. Denominators
  are evacuated PSUM->SBUF immediately so the next i-chunk's AV accumulation
  can reuse the PSUM bank without waiting for the (slow, off-critical-path)
  reciprocal: iq 0-2 reciprocals on DVE, the final iq's on ACT via
  exp(-ln d) right after the last exp call. The reciprocal row is broadcast
  across 64 partitions by a tiny fp16 ones-vector matmul.
- Projection: attn (fp16) slices are the stationary operand with explicit
  ldweights; heads 0/1 interleave on disjoint 64-row PE tiles so their
  matmul streams overlap. (The old fp32r self-loading path cost ~8x more.)
- Optional knob exp_dve_jbs routes the exp of selected j-blocks to the DVE
  as a Schraudolph fp16 bit-trick (tensor_scalar mult+add -> int16 view);
  ~3.6% elementwise sawtooth error that largely averages out in AV.

Emission is software-pipelined one group (iq i-chunk, jb j-block) ahead:
scores(g+1) are issued before AV(g) so the PE computes scores while ACT
evaluates exp(g). The back half of QKV is spread into the early attention
stream; projections fire mid-stream (mb=0) and at the tail (mb=1).

Host-side prep per core: x[b] transposed to channel-major (the PE contracts
over the partition axis), qkv_w column slice for its heads, proj_w
rearranged fp16 for the scrambled-row projection. Host-side unshard:
row-slice concatenation + bias add.
"""

import numpy as np
import ml_dtypes
from contextlib import ExitStack

import concourse.tile as tile
from concourse import bacc, mybir
from concourse.bass_utils import run_bass_kernel_spmd
from concourse.masks import make_identity

B, N, C, H = 2, 2048, 512, 8
D = C // H            # 64
SCALE = D ** -0.5
N_CORES = 8
F32 = mybir.dt.float32
FP16 = mybir.dt.float16
I16 = mybir.dt.int16
EXP = mybir.ActivationFunctionType.Exp
LN = mybir.ActivationFunctionType.Ln
COPY = mybir.ActivationFunctionType.Copy

# Schraudolph fp16 exp: bits = round(s*SCALE * 1024/ln2 + (15360 + C_ADJ))
EXPA = SCALE * 1024.0 / float(np.log(2.0))
EXPB = 15360.0 - 58.0

_programs = {}


def build_program(reps: int = 1, debug: bool = False,
                  do_attn: bool = True, do_proj: bool = True,
                  do_qkv: bool = True, exp_dve_jbs: tuple = (),
                  expb: float = EXPB, recip_act_all: bool = False,
                  exp_half: bool = False, scores_single: bool = False,
                  unroll: int = 1, loop_kw: dict | None = None):
    """Build + compile the SPMD single-core program.

    reps > 1 wraps the whole body in a hardware loop (used only for timing
    calibration). debug=True adds DRAM dumps of intermediates. The do_* /
    exp_dve_jbs knobs build timing-experiment variants.
    """
    nc = bacc.Bacc("TRN2", target_bir_lowering=False, debug=False,
                   num_devices=N_CORES)
    xt = nc.dram_tensor("xt", [C, N], FP16, kind="ExternalInput").ap()
    wqkv = nc.dram_tensor("wqkv", [C, 384], FP16, kind="ExternalInput").ap()
    wp = nc.dram_tensor("wp", [128, 8, C], FP16, kind="ExternalInput").ap()
    part = nc.dram_tensor("part", [512, C], F32, kind="ExternalOutput").ap()
    dbg = {}
    if debug:
        for name, shape in (("d_qT", [128, N]), ("d_kT", [128, N]),
                            ("d_vext", [128, 16 * 130]), ("d_attn", [128, N])):
            dbg[name] = nc.dram_tensor(name, shape, F32, kind="ExternalOutput").ap()

    with tile.TileContext(nc) as tc, ExitStack() as ctx:
        ctx.enter_context(nc.allow_low_precision(reason="fp16 attention kernel"))
        consts = ctx.enter_context(tc.tile_pool(name="consts", bufs=1))
        bigs = ctx.enter_context(tc.tile_pool(name="bigs", bufs=1))
        probs_pool = ctx.enter_context(tc.tile_pool(name="probs", bufs=4))
        small = ctx.enter_context(tc.tile_pool(name="small", bufs=2))
        outp = ctx.enter_context(tc.tile_pool(name="outp", bufs=2))

        ident_f = consts.tile([128, 128], F32)
        make_identity(nc, ident_f[:])
        ident16 = consts.tile([128, 128], FP16)
        nc.vector.tensor_copy(out=ident16[:], in_=ident_f[:])
        ones_f = consts.tile([128, 128], F32)
        nc.vector.memset(ones_f[:], 1.0)
        ones16 = consts.tile([1, 128], FP16)
        nc.vector.tensor_copy(out=ones16[:], in_=ones_f[0:1, :])
        ones_wide = consts.tile([128, 32], FP16)
        nc.vector.tensor_copy(out=ones_wide[:], in_=ones_f[:, 0:32])

        def body():
            # ---- loads -------------------------------------------------
            # weights first (small), then x in 4 n-chunks so the first QKV
            # matmuls start early instead of waiting for the full 2MB.
            wqkv_sb = bigs.tile([128, 4, 384], FP16, tag="wqkv")
            wqkv_v = wqkv.rearrange("(k p) f -> p k f", p=128)
            for k in range(4):
                nc.sync.dma_start(out=wqkv_sb[:, k, :], in_=wqkv_v[:, k, :])
            xt_sb = bigs.tile([128, 4, 4, 512], FP16, tag="xt")
            xt_v = xt.rearrange("(k p) (nb n) -> p k nb n", p=128, nb=4)
            for k in range(4):
                nc.sync.dma_start(out=xt_sb[:, k, 0:2, :], in_=xt_v[:, k, 0:2, :])
            for nb in range(2, 4):
                nc.sync.dma_start(out=xt_sb[:, :, nb, :], in_=xt_v[:, :, nb, :])
            wp_sb = bigs.tile([128, 8, C], FP16, tag="wp")
            nc.sync.dma_start(out=wp_sb[:], in_=wp)

            qT = bigs.tile([128, N], FP16, tag="qT")
            kT = bigs.tile([128, N], FP16, tag="kT")
            vT = bigs.tile([128, N], FP16, tag="vT")
            attn = bigs.tile([128, N], FP16, tag="attn")
            # v in row-major [j, 64+ones | 64+ones] blocks; ones col feeds the
            # softmax-denominator row of the AV matmul.
            vext = bigs.tile([128, 16, 130], FP16, tag="vext")
            vext_cols = vext[:].rearrange("p a (b c) -> p a b c", b=2)
            nc.vector.tensor_copy(
                out=vext_cols[:, :, :, 64],
                in_=ones_wide[:].rearrange("p (a b) -> p a b", a=16))

            # PSUM: scr 2 banks x2 bufs + av0/av1 1 bank each + qkv 2 = 8.
            with tc.tile_pool(name="ps_qkv", bufs=2, space="PSUM") as ps_qkv, \
                 tc.tile_pool(name="ps_scr", bufs=2, space="PSUM") as ps_scr, \
                 tc.tile_pool(name="ps_av", bufs=1, space="PSUM") as ps_av:
                dests = (qT, kT, vT)

                def qkv_f_mm(nb, f, pa, pb, k):
                    w = wqkv_sb[:, k, f * 128:(f + 1) * 128]
                    nc.tensor.ldweights(weights=w)
                    for ps, nbx in ((pa, nb), (pb, nb + 1)):
                        mm = nc.tensor.matmul(
                            ps[:], w, xt_sb[:, k, nbx, :],
                            start=(k == 0), stop=(k == 3))
                        mm.ins.ldweights = False

                def qkv_f_copy(nb, f, pa, pb, copy_eng):
                    for ps, nbx in ((pa, nb), (pb, nb + 1)):
                        dst = dests[f][:, nbx * 512:(nbx + 1) * 512]
                        if copy_eng == "act":
                            nc.scalar.activation(out=dst, in_=ps[:], func=COPY)
                        else:
                            nc.vector.tensor_copy(out=dst, in_=ps[:])

                def qkv_fs(nb, fs, copy_eng):
                    if not do_qkv and nb == 0 and 0 in fs:
                        for t in dests:
                            nc.vector.memset(t[:], 0.00390625)
                    # two n-chunks share each fp16 weight load (LDW reuse)
                    for f in (fs if do_qkv else ()):
                        pa = ps_qkv.tile([128, 512], F32, tag="qkv", name=f"qa{nb}{f}")
                        pb = ps_qkv.tile([128, 512], F32, tag="qkv", name=f"qb{nb}{f}")
                        for k in range(4):
                            qkv_f_mm(nb, f, pa, pb, k)
                        qkv_f_copy(nb, f, pa, pb, copy_eng)

                def v_transpose(jb):
                    pst = ps_qkv.tile([128, 128], FP16, tag="qkv")
                    nc.tensor.transpose(pst[:], vT[:, jb * 128:(jb + 1) * 128], ident16[:])
                    nc.vector.tensor_copy(out=vext[:, jb, 0:64], in_=pst[:, 0:64])
                    nc.vector.tensor_copy(out=vext[:, jb, 65:129], in_=pst[:, 64:128])

                def v_transposes(nb):
                    # transpose these n-chunks of v to row-major via PE
                    for jb in range(4 * nb, 4 * nb + 8):
                        v_transpose(jb)

                def qkv_pair(nb, copy_eng):
                    qkv_fs(nb, (0, 1, 2), copy_eng)
                    v_transposes(nb)

                def scores_g(iq, jb):
                    # scoresT[j, i] for 128 j's x (2 heads x 512 i's); ONE
                    # full-width ldweights holds both heads' K^T block and
                    # the two matmuls run concurrently on disjoint 64-row
                    # PE tiles.
                    scr = ps_scr.tile([128, 1024], F32, tag="scr")
                    nc.tensor.ldweights(weights=kT[:, jb * 128:(jb + 1) * 128])
                    for h in range(1 if scores_single else 2):
                        hp = slice(64 * h, 64 * h + 64)
                        mm = nc.tensor.matmul(
                            scr[:, h * 512:(h + 1) * 512],
                            kT[hp, jb * 128:(jb + 1) * 128],
                            qT[hp, iq * 512:(iq + 1) * 512],
                            start=True, stop=True)
                        mm.ins.ldweights = False
                    # scores_single is only valid with exp_half (which never
                    # reads scr[:, 512:]): isolates MM_B for concurrency
                    # timing.
                    return scr

                def exp_g(scr, jb):
                    pr = probs_pool.tile([128, 1024], FP16, tag="pr")
                    if jb in exp_dve_jbs:
                        nc.vector.tensor_scalar(
                            out=pr[:].bitcast(I16), in0=scr[:],
                            scalar1=float(EXPA), scalar2=float(expb),
                            op0=mybir.AluOpType.mult, op1=mybir.AluOpType.add)
                    elif exp_half:
                        # timing experiment: half the ACT columns (wrong)
                        nc.scalar.activation(out=pr[:, 0:512], in_=scr[:, 0:512],
                                             func=EXP, scale=SCALE)
                    else:
                        nc.scalar.activation(out=pr[:], in_=scr[:], func=EXP,
                                             scale=SCALE)
                    return pr

                def av_g(h, av, pr, jb):
                    vblk = vext[:, jb, 65 * h:65 * h + 65]
                    nc.tensor.ldweights(weights=vblk)
                    prh = pr[:, 0:512] if exp_half else pr[:, h * 512:(h + 1) * 512]
                    mm = nc.tensor.matmul(
                        av[0:65, :], vblk, prh,
                        start=(jb == 0), stop=(jb == 15))
                    mm.ins.ldweights = False

                def evac(h, iq, av):
                    # evacuate av PSUM->SBUF so the bank frees for the next
                    # iq without waiting for the (slow) reciprocal. Both
                    # heads evacuate BEFORE either reciprocal is emitted so
                    # the in-order DVE never holds av1's bank hostage.
                    avs = small.tile([128, 512], F32, tag=f"avs{h}", bufs=2,
                                     name=f"avs{h}_{iq}")
                    nc.vector.tensor_copy(out=avs[0:65, :], in_=av[0:65, :])
                    return avs

                def recip(h, iq, avs):
                    rc = small.tile([1, 512], FP16, tag=f"rc{h}", bufs=2,
                                    name=f"rc{h}_{iq}")
                    if recip_act_all or iq == 3:
                        ld = small.tile([1, 512], F32, tag=f"ld{h}", bufs=2,
                                        name=f"ld{h}_{iq}")
                        nc.scalar.activation(out=ld[:], in_=avs[64:65, :], func=LN)
                        nc.scalar.activation(out=rc[:], in_=ld[:], func=EXP,
                                             scale=-1.0)
                    else:
                        nc.vector.reciprocal(rc[:], avs[64:65, :])
                    return rc

                def finish_norm(h, iq, avs, rc):
                    if iq == 3:
                        # reuse the (evacuated) av bank: keeps the qkv-tag
                        # rotation free at the tail so the next rep's first
                        # QKV PSUM alloc isn't gated on this rep's norm.
                        bc = ps_av.tile([128, 512], F32, tag=f"av{h}",
                                        name=f"bc{h}{iq}")
                    else:
                        bc = ps_qkv.tile([128, 512], F32, tag="qkv",
                                         name=f"bc{h}{iq}")
                    nc.tensor.matmul(bc[0:64, :], ones16[0:1, 0:64], rc[0:1, :],
                                     start=True, stop=True)
                    # The host permutes x's token axis g-major (position
                    # p = mb*1024 + g*128 + m for token i = mb*1024+8m+g),
                    # so this contiguous write leaves attn exactly in the
                    # layout whose per-(mb,g) projection weight slices are
                    # contiguous 128-col ldweights (fast-weight-load path).
                    nc.vector.tensor_mul(
                        attn[64 * h:64 * h + 64, iq * 512:(iq + 1) * 512],
                        avs[0:64, :], bc[0:64, :])

                def proj_mms(mb, pps, gs):
                    # projection for both heads, interleaved so the per-head
                    # matmuls run concurrently on disjoint 64-row PE tiles.
                    # out rows n'=h*256+mb*128+m, contraction c'=(g,d); attn
                    # is g-major so each weight slice is contiguous.
                    glast = 7 if do_proj else 0
                    attn_v = attn.rearrange("p (mb g m) -> p mb g m", mb=2, g=8)
                    for g in gs:
                        # ONE full-width ldweights holds both heads' attn
                        # slice; the two matmuls run concurrently on
                        # disjoint 64-row PE tiles (same trick as scores).
                        nc.tensor.ldweights(weights=attn_v[:, mb, g, :])
                        for h in range(2):
                            hp = slice(64 * h, 64 * h + 64)
                            mm = nc.tensor.matmul(
                                pps[h][:], attn_v[hp, mb, g, :], wp_sb[hp, g, :],
                                start=(g == 0), stop=(g == glast),
                                tile_position=(64 * h, 0))
                            mm.ins.ldweights = False

                def proj_out(mb, pps):
                    for h in range(2):
                        ob = outp.tile([128, 512], F32, tag="ob")
                        nc.vector.tensor_copy(out=ob[:], in_=pps[h][:])
                        # issue the output DMA from the (otherwise idle)
                        # Pool engine: keeping SP's queue load-only lets the
                        # next rep's input DMAs issue during this rep's tail
                        nc.gpsimd.dma_start(
                            out=part.rearrange("(r p) c -> r p c", p=128)[2 * h + mb],
                            in_=ob[:])

                def proj_pair(mb):
                    pps = [ps_qkv.tile([128, 512], F32, tag="qkv",
                                       name=f"pp{h}{mb}") for h in range(2)]
                    proj_mms(mb, pps, range(8 if do_proj else 1))
                    proj_out(mb, pps)

                # Software-pipelined emission: scores of group g+1 are
                # emitted BEFORE av of group g so the static schedule lets
                # the PE run ahead while ACT evaluates exp(g); the back half
                # of QKV is spread into the early attention stream. Norm
                # completion (bc broadcast + divide-multiply) for iq is
                # deferred to mid-(iq+1) so slow reciprocals never stall
                # the in-order PE stream.
                # Dribble plan for the second QKV half: tiny self-contained
                # PE pieces (one ldw+mm, or a transpose, or a PSUM-drain
                # copy) spread through the ACT-bound stream's PE slack.
                # Ordering constraints: v chunks feed transposes; kT chunk
                # nb must be EMITTED before the scores prefetch that reads
                # it (jb8 at idx6, jb12 at idx10); vext jb must be emitted
                # before av reads it (idx jb); q needed by iq2 (idx30).
                drib_units = {}

                def drib_mm(nb, f, k):
                    if (nb, f) not in drib_units:
                        drib_units[(nb, f)] = ps_qkv.tile(
                            [128, 512], F32, tag="qkv", name=f"qu{nb}{f}")
                    pu = drib_units[(nb, f)]
                    w = wqkv_sb[:, k, f * 128:(f + 1) * 128]
                    nc.tensor.ldweights(weights=w)
                    mm = nc.tensor.matmul(pu[:], w, xt_sb[:, k, nb, :],
                                          start=(k == 0), stop=(k == 3))
                    mm.ins.ldweights = False

                def drib_copy(nb, f):
                    pu = drib_units.pop((nb, f))
                    nc.vector.tensor_copy(
                        out=dests[f][:, nb * 512:(nb + 1) * 512], in_=pu[:])

                DRIB = {}
                if do_qkv:
                    def M(nb, f, k):
                        return ("mm", nb, f, k)
                    # Hand-scheduled: dribble(idx) runs at the END of group
                    # idx's emission, so a piece needed by group i's own
                    # reads must sit at idx <= i-1: vext t(jb) before av at
                    # idx jb; kT chunk nb2/nb3 before the scores PREFETCH
                    # (emitted at idx jb-2) that reads it; vT chunks before
                    # their transposes; q chunks before iq2 (idx 30).
                    DRIB = {
                        0: [M(2, 2, 0), M(2, 2, 1)],
                        1: [M(2, 2, 2), M(2, 2, 3)],
                        2: [("cp", 2, 2), ("t", 8)],
                        3: [M(2, 1, 0), M(2, 1, 1)],
                        4: [M(2, 1, 2), M(2, 1, 3)],
                        5: [("cp", 2, 1), ("t", 9)],
                        6: [M(3, 1, 0), M(3, 1, 1)],
                        7: [M(3, 1, 2), M(3, 1, 3)],
                        8: [("cp", 3, 1), ("t", 10)],
                        9: [M(3, 2, 0), M(3, 2, 1), ("t", 11)],
                        10: [M(3, 2, 2), M(3, 2, 3), ("cp", 3, 2)],
                        11: [("t", 12), ("t", 13)],
                        12: [("t", 14), ("t", 15)],
                        13: [M(2, 0, 0), M(2, 0, 1)],
                        14: [M(2, 0, 2), M(2, 0, 3)],
                        15: [("cp", 2, 0)],
                        16: [M(3, 0, 0), M(3, 0, 1)],
                        17: [M(3, 0, 2), M(3, 0, 3)],
                        18: [("cp", 3, 0)],
                    }

                def dribble(idx):
                    for piece in DRIB.get(idx, ()):
                        if piece[0] == "mm":
                            drib_mm(piece[1], piece[2], piece[3])
                        elif piece[0] == "cp":
                            drib_copy(piece[1], piece[2])
                        else:
                            v_transpose(piece[1])

                if do_attn:
                    groups = [(iq, jb) for iq in range(4) for jb in range(16)]
                    navs = {}
                    # prefix: q,k then the first two score groups, then v.
                    # Prefix copies ride ACT: the next rep's stream start
                    # depends on them, and ACT's tail backlog at the rep
                    # boundary (~2.5us) is far shorter than DVE's (~12us).
                    qkv_fs(0, (0, 1), copy_eng="act")
                    scrs = [scores_g(*groups[0]), scores_g(*groups[1])]
                    qkv_fs(0, (2,), copy_eng="act")
                    v_transposes(0)
                    for idx, (iq, jb) in enumerate(groups):
                        if jb == 0:
                            av0 = ps_av.tile([128, 512], F32, tag="av0",
                                             name=f"av0_{iq}")
                            av1 = ps_av.tile([128, 512], F32, tag="av1",
                                             name=f"av1_{iq}")
                        pr = exp_g(scrs[idx], jb)
                        if idx + 2 < len(groups):
                            scrs.append(scores_g(*groups[idx + 2]))
                        av_g(0, av0, pr, jb)
                        av_g(1, av1, pr, jb)
                        dribble(idx)
                        if jb == 15:
                            avs0 = evac(0, iq, av0)
                            avs1 = evac(1, iq, av1)
                            if iq < 3:
                                navs[(0, iq)] = (avs0, recip(0, iq, avs0))
                                navs[(1, iq)] = (avs1, recip(1, iq, avs1))
                        if jb == 11 and iq >= 1:
                            for h in range(2):
                                finish_norm(h, iq - 1, *navs.pop((h, iq - 1)))
                        if (iq, jb) == (2, 13):
                            proj_pair(0)
                        if (iq, jb) == (3, 13) and do_proj:
                            # attn is g-major: proj mb=1's g 0-3 touch only
                            # iq2's rows (normalized at (3,11)) - start the
                            # second projection before the last iq finishes.
                            pps1 = [ps_qkv.tile([128, 512], F32, tag="qkv",
                                                name=f"pp{h}1") for h in range(2)]
                            proj_mms(1, pps1, range(0, 4))
                    # tail: overlap head-0 norm with head-1's reciprocal.
                    rc0 = recip(0, 3, avs0)
                    finish_norm(0, 3, avs0, rc0)
                    rc1 = recip(1, 3, avs1)
                    finish_norm(1, 3, avs1, rc1)
                    if do_proj:
                        proj_mms(1, pps1, range(4, 8))
                        proj_out(1, pps1)
                    else:
                        proj_pair(1)
                else:
                    qkv_pair(0, copy_eng="act")
                    qkv_pair(2, copy_eng="dve")
                    nc.vector.memset(attn[:], 0.00390625)
                    proj_pair(0)
                    proj_pair(1)
            if debug:
                for name, t in (("d_qT", qT), ("d_kT", kT), ("d_attn", attn)):
                    sb = outp.tile([128, N], F32, tag="dbg")
                    nc.vector.tensor_copy(out=sb[:], in_=t[:])
                    nc.sync.dma_start(out=dbg[name], in_=sb[:])
                sb = outp.tile([128, 16 * 130], F32, tag="dbg")
                nc.vector.tensor_copy(out=sb[:], in_=vext[:].rearrange("p a b -> p (a b)"))
                nc.sync.dma_start(out=dbg["d_vext"], in_=sb[:])

        if reps == 1:
            for _ in range(unroll):
                body()
        else:
            assert reps % unroll == 0
            with tc.For_i(0, reps // unroll, 1, **(loop_kw or {})):
                for _ in range(unroll):
                    body()

    nc.compile()
    return nc


def _get_program(reps: int = 1, debug: bool = False, **kw):
    key = (reps, debug, repr(sorted(kw.items())))
    if key not in _programs:
        _programs[key] = build_program(reps, debug, **kw)
    return _programs[key]


def _token_perm():
    """Device token order: position mb*1024 + g*128 + m holds token
    i = mb*1024 + 8m + g. Softmax is order-invariant over j and each
    column's softmax is complete, so permuting the token axis on the host
    makes every device-side access contiguous AND leaves attn g-major so
    projection weight loads hit the fast contiguous ldweights path."""
    i = np.arange(N).reshape(2, 128, 8)            # [mb, m, g]
    return i.transpose(0, 2, 1).reshape(N)         # [mb, g, m] -> flat


def _in_maps(x, qkv_w, proj_w):
    perm = _token_perm()
    wp_arr = np.ascontiguousarray(
        np.tile(proj_w.reshape(8, 64, C).transpose(1, 0, 2),
                (2, 1, 1))).astype(np.float16)
    maps = []
    for c in range(N_CORES):
        b, p = divmod(c, 4)
        xt = np.ascontiguousarray(x[b][perm].T.astype(np.float16))
        wqkv = np.ascontiguousarray(np.concatenate(
            [qkv_w[:, t * C + p * 128: t * C + p * 128 + 128] for t in range(3)],
            axis=1).astype(np.float16))
        maps.append({"xt": xt, "wqkv": wqkv, "wp": wp_arr})
    return maps


def kernel(**inputs) -> np.ndarray:
    x = np.asarray(inputs["x"], np.float32)
    qkv_w = np.asarray(inputs["qkv_w"], np.float32)
    proj_w = np.asarray(inputs["proj_w"], np.float32)
    proj_b = np.asarray(inputs["proj_b"], np.float32)

    nc = _get_program()
    res = run_bass_kernel_spmd(nc, _in_maps(x, qkv_w, proj_w),
                               core_ids=list(range(N_CORES)))
    out = np.empty((B, N, C), np.float32)
    for c in range(N_CORES):
        b, p = divmod(c, 4)
        out[b, p * 512:(p + 1) * 512, :] = res.results[c]["part"]
    out += proj_b
    return out


# revision 43
# speedup vs baseline: 1.0359x; 1.0065x over previous
"""Trainium2 Bass kernel for nn_Attention_83141976916236.

Reference computation (B=2, N=2048, C=512, H=8, D=64):
    qkv = x @ qkv_w                       -> split to q, k, v per head
    att_h = softmax(q_h k_h^T / sqrt(D)) v_h        (per batch b, head h)
    out  = reshape_no_transpose(att) @ proj_w + proj_b

Key structural fact: the reference reshapes (B,H,N,D) -> (B,N,C) WITHOUT
transposing, so output row n' = h*256 + n//8 with channel c' = (n%8)*64 + d.
Every output row therefore depends on exactly ONE head: with heads sharded
across cores, each core produces a disjoint slice of output rows and the
host-side unshard is a pure concatenation (no cross-core reduction).

Sharding (8 cores): core c handles batch b = c//4 and heads (2p, 2p+1) where
p = c%4. Per core, everything runs in fp16 on the PE (fp32 PSUM accumulate):

- QKV: explicit 128-col ldweights shared by two 512-wide moving chunks.
- Scores: ONE full 128x128 ldweights loads BOTH heads' K^T block (head h
  occupies array rows 64h..64h+63); the two per-head matmuls then address
  disjoint 64-row PE tiles (tile_position (0,0)/(64,0)) and execute
  CONCURRENTLY on the systolic array, halving score time vs per-head loads.
- Softmax: scoresT kept [j, i]; exp on ACT (table set natural_log_exp: Exp,
  Ln and Copy live in one set so no ACT table reloads). An appended ones
  column in the AV weights yields the denominator row for free. Denominators
  are evacuated PSUM->SBUF immediately so the next i-chunk's AV accumulation
  can reuse the PSUM bank without waiting for the (slow, off-critical-path)
  reciprocal: iq 0-2 reciprocals on DVE, the final iq's on ACT via
  exp(-ln d) right after the last exp call. The reciprocal row is broadcast
  across 64 partitions by a tiny fp16 ones-vector matmul.
- Projection: attn (fp16) slices are the stationary operand with explicit
  ldweights; heads 0/1 interleave on disjoint 64-row PE tiles so their
  matmul streams overlap. (The old fp32r self-loading path cost ~8x more.)
- Optional knob exp_dve_jbs routes the exp of selected j-blocks to the DVE
  as a Schraudolph fp16 bit-trick (tensor_scalar mult+add -> int16 view);
  ~3.6% elementwise sawtooth error that largely averages out in AV.

Emission is software-pipelined one group (iq i-chunk, jb j-block) ahead:
scores(g+1) are issued before AV(g) so the PE computes scores while ACT
evaluates exp(g). The back half of QKV is spread into the early attention
stream; projections fire mid-stream (mb=0) and at the tail (mb=1).

Host-side prep per core: x[b] transposed to channel-major (the PE contracts
over the partition axis), qkv_w column slice for its heads, proj_w
rearranged fp16 for the scrambled-row projection. Host-side unshard:
row-slice concatenation + bias add.
"""

import numpy as np
import ml_dtypes
from contextlib import ExitStack

import concourse.tile as tile
from concourse import bacc, mybir
from concourse.bass_utils import run_bass_kernel_spmd
from concourse.masks import make_identity

B, N, C, H = 2, 2048, 512, 8
D = C // H            # 64
SCALE = D ** -0.5
N_CORES = 8
F32 = mybir.dt.float32
FP16 = mybir.dt.float16
I16 = mybir.dt.int16
EXP = mybir.ActivationFunctionType.Exp
LN = mybir.ActivationFunctionType.Ln
COPY = mybir.ActivationFunctionType.Copy

# Schraudolph fp16 exp: bits = round(s*SCALE * 1024/ln2 + (15360 + C_ADJ))
EXPA = SCALE * 1024.0 / float(np.log(2.0))
EXPB = 15360.0 - 58.0

_programs = {}


def build_program(reps: int = 1, debug: bool = False,
                  do_attn: bool = True, do_proj: bool = True,
                  do_qkv: bool = True, exp_dve_jbs: tuple = (),
                  expb: float = EXPB, recip_act_all: bool = False,
                  exp_half: bool = False, scores_single: bool = False,
                  unroll: int = 1, loop_kw: dict | None = None):
    """Build + compile the SPMD single-core program.

    reps > 1 wraps the whole body in a hardware loop (used only for timing
    calibration). debug=True adds DRAM dumps of intermediates. The do_* /
    exp_dve_jbs knobs build timing-experiment variants.
    """
    nc = bacc.Bacc("TRN2", target_bir_lowering=False, debug=False,
                   num_devices=N_CORES)
    xt = nc.dram_tensor("xt", [C, N], FP16, kind="ExternalInput").ap()
    wqkv = nc.dram_tensor("wqkv", [C, 384], FP16, kind="ExternalInput").ap()
    wp = nc.dram_tensor("wp", [128, 8, C], FP16, kind="ExternalInput").ap()
    part = nc.dram_tensor("part", [512, C], F32, kind="ExternalOutput").ap()
    dbg = {}
    if debug:
        for name, shape in (("d_qT", [128, N]), ("d_kT", [128, N]),
                            ("d_vext", [128, 16 * 130]), ("d_attn", [128, N])):
            dbg[name] = nc.dram_tensor(name, shape, F32, kind="ExternalOutput").ap()

    with tile.TileContext(nc) as tc, ExitStack() as ctx:
        ctx.enter_context(nc.allow_low_precision(reason="fp16 attention kernel"))
        consts = ctx.enter_context(tc.tile_pool(name="consts", bufs=1))
        bigs = ctx.enter_context(tc.tile_pool(name="bigs", bufs=1))
        probs_pool = ctx.enter_context(tc.tile_pool(name="probs", bufs=4))
        small = ctx.enter_context(tc.tile_pool(name="small", bufs=2))
        outp = ctx.enter_context(tc.tile_pool(name="outp", bufs=2))

        ident_f = consts.tile([128, 128], F32)
        make_identity(nc, ident_f[:])
        ident16 = consts.tile([128, 128], FP16)
        nc.vector.tensor_copy(out=ident16[:], in_=ident_f[:])
        ones_f = consts.tile([128, 128], F32)
        nc.vector.memset(ones_f[:], 1.0)
        ones16 = consts.tile([1, 128], FP16)
        nc.vector.tensor_copy(out=ones16[:], in_=ones_f[0:1, :])
        ones_wide = consts.tile([128, 32], FP16)
        nc.vector.tensor_copy(out=ones_wide[:], in_=ones_f[:, 0:32])

        def body():
            # ---- loads -------------------------------------------------
            # weights first (small), then x in 4 n-chunks so the first QKV
            # matmuls start early instead of waiting for the full 2MB.
            # bufs=2 on the DMA-fed tiles: the next rep's loads start while
            # this rep still reads the previous buffer.
            wqkv_sb = bigs.tile([128, 4, 384], FP16, tag="wqkv", bufs=2)
            wqkv_v = wqkv.rearrange("(k p) f -> p k f", p=128)
            for k in range(4):
                nc.sync.dma_start(out=wqkv_sb[:, k, :], in_=wqkv_v[:, k, :])
            xt_sb = bigs.tile([128, 4, 4, 512], FP16, tag="xt", bufs=2)
            xt_v = xt.rearrange("(k p) (nb n) -> p k nb n", p=128, nb=4)
            for k in range(4):
                nc.sync.dma_start(out=xt_sb[:, k, 0:2, :], in_=xt_v[:, k, 0:2, :])
            for nb in range(2, 4):
                nc.sync.dma_start(out=xt_sb[:, :, nb, :], in_=xt_v[:, :, nb, :])
            wp_sb = bigs.tile([128, 8, C], FP16, tag="wp", bufs=2)
            nc.sync.dma_start(out=wp_sb[:], in_=wp)

            qT = bigs.tile([128, N], FP16, tag="qT")
            kT = bigs.tile([128, N], FP16, tag="kT")
            vT = bigs.tile([128, N], FP16, tag="vT")
            attn = bigs.tile([128, N], FP16, tag="attn")
            # v in row-major [j, 64+ones | 64+ones] blocks; ones col feeds the
            # softmax-denominator row of the AV matmul.
            vext = bigs.tile([128, 16, 130], FP16, tag="vext")
            vext_cols = vext[:].rearrange("p a (b c) -> p a b c", b=2)
            nc.vector.tensor_copy(
                out=vext_cols[:, :, :, 64],
                in_=ones_wide[:].rearrange("p (a b) -> p a b", a=16))

            # PSUM: scr 2 banks x2 bufs + av0/av1 1 bank each + qkv 2 = 8.
            with tc.tile_pool(name="ps_qkv", bufs=2, space="PSUM") as ps_qkv, \
                 tc.tile_pool(name="ps_scr", bufs=2, space="PSUM") as ps_scr, \
                 tc.tile_pool(name="ps_av", bufs=1, space="PSUM") as ps_av:
                dests = (qT, kT, vT)

                def qkv_f_mm(nb, f, pa, pb, k):
                    w = wqkv_sb[:, k, f * 128:(f + 1) * 128]
                    nc.tensor.ldweights(weights=w)
                    for ps, nbx in ((pa, nb), (pb, nb + 1)):
                        mm = nc.tensor.matmul(
                            ps[:], w, xt_sb[:, k, nbx, :],
                            start=(k == 0), stop=(k == 3))
                        mm.ins.ldweights = False

                def qkv_f_copy(nb, f, pa, pb, copy_eng):
                    for ps, nbx in ((pa, nb), (pb, nb + 1)):
                        dst = dests[f][:, nbx * 512:(nbx + 1) * 512]
                        if copy_eng == "act":
                            nc.scalar.activation(out=dst, in_=ps[:], func=COPY)
                        else:
                            nc.vector.tensor_copy(out=dst, in_=ps[:])

                def qkv_fs(nb, fs, copy_eng):
                    if not do_qkv and nb == 0 and 0 in fs:
                        for t in dests:
                            nc.vector.memset(t[:], 0.00390625)
                    # two n-chunks share each fp16 weight load (LDW reuse)
                    for f in (fs if do_qkv else ()):
                        pa = ps_qkv.tile([128, 512], F32, tag="qkv", name=f"qa{nb}{f}")
                        pb = ps_qkv.tile([128, 512], F32, tag="qkv", name=f"qb{nb}{f}")
                        for k in range(4):
                            qkv_f_mm(nb, f, pa, pb, k)
                        qkv_f_copy(nb, f, pa, pb, copy_eng)

                def v_transpose(jb):
                    pst = ps_qkv.tile([128, 128], FP16, tag="qkv")
                    nc.tensor.transpose(pst[:], vT[:, jb * 128:(jb + 1) * 128], ident16[:])
                    nc.vector.tensor_copy(out=vext[:, jb, 0:64], in_=pst[:, 0:64])
                    nc.vector.tensor_copy(out=vext[:, jb, 65:129], in_=pst[:, 64:128])

                def v_transposes(nb):
                    # transpose these n-chunks of v to row-major via PE
                    for jb in range(4 * nb, 4 * nb + 8):
                        v_transpose(jb)

                def qkv_pair(nb, copy_eng):
                    qkv_fs(nb, (0, 1, 2), copy_eng)
                    v_transposes(nb)

                def scores_g(iq, jb):
                    # scoresT[j, i] for 128 j's x (2 heads x 512 i's); ONE
                    # full-width ldweights holds both heads' K^T block and
                    # the two matmuls run concurrently on disjoint 64-row
                    # PE tiles.
                    scr = ps_scr.tile([128, 1024], F32, tag="scr")
                    nc.tensor.ldweights(weights=kT[:, jb * 128:(jb + 1) * 128])
                    for h in range(1 if scores_single else 2):
                        hp = slice(64 * h, 64 * h + 64)
                        mm = nc.tensor.matmul(
                            scr[:, h * 512:(h + 1) * 512],
                            kT[hp, jb * 128:(jb + 1) * 128],
                            qT[hp, iq * 512:(iq + 1) * 512],
                            start=True, stop=True)
                        mm.ins.ldweights = False
                    # scores_single is only valid with exp_half (which never
                    # reads scr[:, 512:]): isolates MM_B for concurrency
                    # timing.
                    return scr

                def exp_g(scr, jb):
                    pr = probs_pool.tile([128, 1024], FP16, tag="pr")
                    if jb in exp_dve_jbs:
                        nc.vector.tensor_scalar(
                            out=pr[:].bitcast(I16), in0=scr[:],
                            scalar1=float(EXPA), scalar2=float(expb),
                            op0=mybir.AluOpType.mult, op1=mybir.AluOpType.add)
                    elif exp_half:
                        # timing experiment: half the ACT columns (wrong)
                        nc.scalar.activation(out=pr[:, 0:512], in_=scr[:, 0:512],
                                             func=EXP, scale=SCALE)
                    else:
                        nc.scalar.activation(out=pr[:], in_=scr[:], func=EXP,
                                             scale=SCALE)
                    return pr

                def av_g(h, av, pr, jb):
                    vblk = vext[:, jb, 65 * h:65 * h + 65]
                    nc.tensor.ldweights(weights=vblk)
                    prh = pr[:, 0:512] if exp_half else pr[:, h * 512:(h + 1) * 512]
                    mm = nc.tensor.matmul(
                        av[0:65, :], vblk, prh,
                        start=(jb == 0), stop=(jb == 15))
                    mm.ins.ldweights = False

                def evac(h, iq, av):
                    # evacuate av PSUM->SBUF so the bank frees for the next
                    # iq without waiting for the (slow) reciprocal. Both
                    # heads evacuate BEFORE either reciprocal is emitted so
                    # the in-order DVE never holds av1's bank hostage.
                    avs = small.tile([128, 512], F32, tag=f"avs{h}", bufs=2,
                                     name=f"avs{h}_{iq}")
                    nc.vector.tensor_copy(out=avs[0:65, :], in_=av[0:65, :])
                    return avs

                def recip(h, iq, avs):
                    rc = small.tile([1, 512], FP16, tag=f"rc{h}", bufs=2,
                                    name=f"rc{h}_{iq}")
                    if recip_act_all or iq == 3:
                        ld = small.tile([1, 512], F32, tag=f"ld{h}", bufs=2,
                                        name=f"ld{h}_{iq}")
                        nc.scalar.activation(out=ld[:], in_=avs[64:65, :], func=LN)
                        nc.scalar.activation(out=rc[:], in_=ld[:], func=EXP,
                                             scale=-1.0)
                    else:
                        nc.vector.reciprocal(rc[:], avs[64:65, :])
                    return rc

                def finish_norm(h, iq, avs, rc):
                    if iq == 3:
                        # reuse the (evacuated) av bank: keeps the qkv-tag
                        # rotation free at the tail so the next rep's first
                        # QKV PSUM alloc isn't gated on this rep's norm.
                        bc = ps_av.tile([128, 512], F32, tag=f"av{h}",
                                        name=f"bc{h}{iq}")
                    else:
                        bc = ps_qkv.tile([128, 512], F32, tag="qkv",
                                         name=f"bc{h}{iq}")
                    nc.tensor.matmul(bc[0:64, :], ones16[0:1, 0:64], rc[0:1, :],
                                     start=True, stop=True)
                    # The host permutes x's token axis g-major (position
                    # p = mb*1024 + g*128 + m for token i = mb*1024+8m+g),
                    # so this contiguous write leaves attn exactly in the
                    # layout whose per-(mb,g) projection weight slices are
                    # contiguous 128-col ldweights (fast-weight-load path).
                    nc.vector.tensor_mul(
                        attn[64 * h:64 * h + 64, iq * 512:(iq + 1) * 512],
                        avs[0:64, :], bc[0:64, :])

                def proj_mms(mb, pps, gs):
                    # projection for both heads, interleaved so the per-head
                    # matmuls run concurrently on disjoint 64-row PE tiles.
                    # out rows n'=h*256+mb*128+m, contraction c'=(g,d); attn
                    # is g-major so each weight slice is contiguous.
                    glast = 7 if do_proj else 0
                    attn_v = attn.rearrange("p (mb g m) -> p mb g m", mb=2, g=8)
                    for g in gs:
                        # ONE full-width ldweights holds both heads' attn
                        # slice; the two matmuls run concurrently on
                        # disjoint 64-row PE tiles (same trick as scores).
                        nc.tensor.ldweights(weights=attn_v[:, mb, g, :])
                        for h in range(2):
                            hp = slice(64 * h, 64 * h + 64)
                            mm = nc.tensor.matmul(
                                pps[h][:], attn_v[hp, mb, g, :], wp_sb[hp, g, :],
                                start=(g == 0), stop=(g == glast),
                                tile_position=(64 * h, 0))
                            mm.ins.ldweights = False

                def proj_out(mb, pps):
                    for h in range(2):
                        ob = outp.tile([128, 512], F32, tag="ob")
                        nc.vector.tensor_copy(out=ob[:], in_=pps[h][:])
                        # issue the output DMA from the (otherwise idle)
                        # Pool engine: keeping SP's queue load-only lets the
                        # next rep's input DMAs issue during this rep's tail
                        nc.gpsimd.dma_start(
                            out=part.rearrange("(r p) c -> r p c", p=128)[2 * h + mb],
                            in_=ob[:])

                def proj_pair(mb):
                    pps = [ps_qkv.tile([128, 512], F32, tag="qkv",
                                       name=f"pp{h}{mb}") for h in range(2)]
                    proj_mms(mb, pps, range(8 if do_proj else 1))
                    proj_out(mb, pps)

                # Software-pipelined emission: scores of group g+1 are
                # emitted BEFORE av of group g so the static schedule lets
                # the PE run ahead while ACT evaluates exp(g); the back half
                # of QKV is spread into the early attention stream. Norm
                # completion (bc broadcast + divide-multiply) for iq is
                # deferred to mid-(iq+1) so slow reciprocals never stall
                # the in-order PE stream.
                # Dribble plan for the second QKV half: tiny self-contained
                # PE pieces (one ldw+mm, or a transpose, or a PSUM-drain
                # copy) spread through the ACT-bound stream's PE slack.
                # Ordering constraints: v chunks feed transposes; kT chunk
                # nb must be EMITTED before the scores prefetch that reads
                # it (jb8 at idx6, jb12 at idx10); vext jb must be emitted
                # before av reads it (idx jb); q needed by iq2 (idx30).
                drib_units = {}

                def drib_mm(nb, f, k):
                    if (nb, f) not in drib_units:
                        drib_units[(nb, f)] = ps_qkv.tile(
                            [128, 512], F32, tag="qkv", name=f"qu{nb}{f}")
                    pu = drib_units[(nb, f)]
                    w = wqkv_sb[:, k, f * 128:(f + 1) * 128]
                    nc.tensor.ldweights(weights=w)
                    mm = nc.tensor.matmul(pu[:], w, xt_sb[:, k, nb, :],
                                          start=(k == 0), stop=(k == 3))
                    mm.ins.ldweights = False

                def drib_copy(nb, f):
                    pu = drib_units.pop((nb, f))
                    nc.vector.tensor_copy(
                        out=dests[f][:, nb * 512:(nb + 1) * 512], in_=pu[:])

                DRIB = {}
                if do_qkv:
                    def M(nb, f, k):
                        return ("mm", nb, f, k)
                    # Hand-scheduled: dribble(idx) runs at the END of group
                    # idx's emission, so a piece needed by group i's own
                    # reads must sit at idx <= i-1: vext t(jb) before av at
                    # idx jb; kT chunk nb2/nb3 before the scores PREFETCH
                    # (emitted at idx jb-2) that reads it; vT chunks before
                    # their transposes; q chunks before iq2 (idx 30).
                    DRIB = {
                        0: [M(2, 2, 0), M(2, 2, 1)],
                        1: [M(2, 2, 2), M(2, 2, 3)],
                        2: [("cp", 2, 2), ("t", 8)],
                        3: [M(2, 1, 0), M(2, 1, 1)],
                        4: [M(2, 1, 2), M(2, 1, 3)],
                        5: [("cp", 2, 1), ("t", 9)],
                        6: [M(3, 1, 0), M(3, 1, 1)],
                        7: [M(3, 1, 2), M(3, 1, 3)],
                        8: [("cp", 3, 1), ("t", 10)],
                        9: [M(3, 2, 0), M(3, 2, 1), ("t", 11)],
                        10: [M(3, 2, 2), M(3, 2, 3), ("cp", 3, 2)],
                        11: [("t", 12), ("t", 13)],
                        12: [("t", 14), ("t", 15)],
                        13: [M(2, 0, 0), M(2, 0, 1)],
                        14: [M(2, 0, 2), M(2, 0, 3)],
                        15: [("cp", 2, 0)],
                        16: [M(3, 0, 0), M(3, 0, 1)],
                        17: [M(3, 0, 2), M(3, 0, 3)],
                        18: [("cp", 3, 0)],
                    }

                def dribble(idx):
                    for piece in DRIB.get(idx, ()):
                        if piece[0] == "mm":
                            drib_mm(piece[1], piece[2], piece[3])
                        elif piece[0] == "cp":
                            drib_copy(piece[1], piece[2])
                        else:
                            v_transpose(piece[1])

                if do_attn:
                    groups = [(iq, jb) for iq in range(4) for jb in range(16)]
                    navs = {}
                    # prefix: q,k then the first two score groups, then v.
                    # Prefix copies ride ACT: the next rep's stream start
                    # depends on them, and ACT's tail backlog at the rep
                    # boundary (~2.5us) is far shorter than DVE's (~12us).
                    qkv_fs(0, (0, 1), copy_eng="act")
                    scrs = [scores_g(*groups[0]), scores_g(*groups[1])]
                    qkv_fs(0, (2,), copy_eng="act")
                    v_transposes(0)
                    for idx, (iq, jb) in enumerate(groups):
                        if jb == 0:
                            av0 = ps_av.tile([128, 512], F32, tag="av0",
                                             name=f"av0_{iq}")
                            av1 = ps_av.tile([128, 512], F32, tag="av1",
                                             name=f"av1_{iq}")
                        pr = exp_g(scrs[idx], jb)
                        if idx + 2 < len(groups):
                            scrs.append(scores_g(*groups[idx + 2]))
                        av_g(0, av0, pr, jb)
                        av_g(1, av1, pr, jb)
                        dribble(idx)
                        if jb == 15:
                            avs0 = evac(0, iq, av0)
                            avs1 = evac(1, iq, av1)
                            if iq < 3:
                                navs[(0, iq)] = (avs0, recip(0, iq, avs0))
                                navs[(1, iq)] = (avs1, recip(1, iq, avs1))
                        if jb == 11 and iq >= 1:
                            for h in range(2):
                                finish_norm(h, iq - 1, *navs.pop((h, iq - 1)))
                        if (iq, jb) == (2, 13):
                            proj_pair(0)
                        if (iq, jb) == (3, 13) and do_proj:
                            # attn is g-major: proj mb=1's g 0-3 touch only
                            # iq2's rows (normalized at (3,11)) - start the
                            # second projection before the last iq finishes.
                            pps1 = [ps_qkv.tile([128, 512], F32, tag="qkv",
                                                name=f"pp{h}1") for h in range(2)]
                            proj_mms(1, pps1, range(0, 4))
                    # tail: overlap head-0 norm with head-1's reciprocal.
                    rc0 = recip(0, 3, avs0)
                    finish_norm(0, 3, avs0, rc0)
                    rc1 = recip(1, 3, avs1)
                    finish_norm(1, 3, avs1, rc1)
                    if do_proj:
                        proj_mms(1, pps1, range(4, 8))
                        proj_out(1, pps1)
                    else:
                        proj_pair(1)
                else:
                    qkv_pair(0, copy_eng="act")
                    qkv_pair(2, copy_eng="dve")
                    nc.vector.memset(attn[:], 0.00390625)
                    proj_pair(0)
                    proj_pair(1)
            if debug:
                for name, t in (("d_qT", qT), ("d_kT", kT), ("d_attn", attn)):
                    sb = outp.tile([128, N], F32, tag="dbg")
                    nc.vector.tensor_copy(out=sb[:], in_=t[:])
                    nc.sync.dma_start(out=dbg[name], in_=sb[:])
                sb = outp.tile([128, 16 * 130], F32, tag="dbg")
                nc.vector.tensor_copy(out=sb[:], in_=vext[:].rearrange("p a b -> p (a b)"))
                nc.sync.dma_start(out=dbg["d_vext"], in_=sb[:])

        if reps == 1:
            for _ in range(unroll):
                body()
        else:
            assert reps % unroll == 0
            with tc.For_i(0, reps // unroll, 1, **(loop_kw or {})):
                for _ in range(unroll):
                    body()

    nc.compile()
    return nc


def _get_program(reps: int = 1, debug: bool = False, **kw):
    key = (reps, debug, repr(sorted(kw.items())))
    if key not in _programs:
        _programs[key] = build_program(reps, debug, **kw)
    return _programs[key]


def _token_perm():
    """Device token order: position mb*1024 + g*128 + m holds token
    i = mb*1024 + 8m + g. Softmax is order-invariant over j and each
    column's softmax is complete, so permuting the token axis on the host
    makes every device-side access contiguous AND leaves attn g-major so
    projection weight loads hit the fast contiguous ldweights path."""
    i = np.arange(N).reshape(2, 128, 8)            # [mb, m, g]
    return i.transpose(0, 2, 1).reshape(N)         # [mb, g, m] -> flat


def _in_maps(x, qkv_w, proj_w):
    perm = _token_perm()
    wp_arr = np.ascontiguousarray(
        np.tile(proj_w.reshape(8, 64, C).transpose(1, 0, 2),
                (2, 1, 1))).astype(np.float16)
    maps = []
    for c in range(N_CORES):
        b, p = divmod(c, 4)
        xt = np.ascontiguousarray(x[b][perm].T.astype(np.float16))
        wqkv = np.ascontiguousarray(np.concatenate(
            [qkv_w[:, t * C + p * 128: t * C + p * 128 + 128] for t in range(3)],
            axis=1).astype(np.float16))
        maps.append({"xt": xt, "wqkv": wqkv, "wp": wp_arr})
    return maps


def kernel(**inputs) -> np.ndarray:
    x = np.asarray(inputs["x"], np.float32)
    qkv_w = np.asarray(inputs["qkv_w"], np.float32)
    proj_w = np.asarray(inputs["proj_w"], np.float32)
    proj_b = np.asarray(inputs["proj_b"], np.float32)

    nc = _get_program()
    res = run_bass_kernel_spmd(nc, _in_maps(x, qkv_w, proj_w),
                               core_ids=list(range(N_CORES)))
    out = np.empty((B, N, C), np.float32)
    for c in range(N_CORES):
        b, p = divmod(c, 4)
        out[b, p * 512:(p + 1) * 512, :] = res.results[c]["part"]
    out += proj_b
    return out
